# revision 31
# baseline (speedup 1.0000x reference)
"""Trainium2 Bass kernel for causal self-attention (B=2, S=2048, D=1024, H=16).

Sharding: 8 cores = 2 batches x 4 head-groups (4 heads / 256 channels each).
Each core computes the qkv projection for its head block, causal attention for
its 4 heads, and a partial output projection (contraction over its 256
channels). The host sums the 4 partials per batch and adds b_out at gather.

Device dataflow (bf16 matmuls, fp32 accumulation):
  - x is fed pre-transposed (xT [D,S]) so every matmul has its contraction on
    partitions with no on-device transposes.
  - Q^T/K^T computed channel-major [ch, t]; V token-major [t, j] with a ones
    column appended -> the attention matmul produces numerator rows 0..63 and
    the softmax denominator in PSUM row 64 in one accumulation group.
  - scores computed transposed (k on partitions, q on free) so exp/mask/AV
    all chain without transposes; softmax needs no max-subtraction (scores
    are O(1) by construction) and division is deferred past the AV matmul.
  - causal: only k-tiles at/below the diagonal are computed; the 4 diagonal
    tiles per q-chunk restrict to their valid column range and apply a
    precomputed multiplicative 0/1 mask.
  - head pairs are packed into the 128-row PE array (base partitions 0/64).
"""

import sys

if "/opt/trn_rl_repo" not in sys.path:
    sys.path.insert(0, "/opt/trn_rl_repo")

import numpy as np
import ml_dtypes

import concourse.mybir as mybir
import concourse.tile as tile
from concourse import bacc

B, S, D, H, DK = 2, 2048, 1024, 16, 64
N_CORES = 8
HPC = 4  # heads per core
DH = HPC * DK  # 256 channels per core
P = 128
QC = 512  # q-chunk width
NQC = S // QC  # 4
NKT = S // P  # 16 k-tiles
DT = D // P  # 8 d-tiles
SCALE = 1.0 / np.sqrt(DK)

BF16 = mybir.dt.bfloat16
F32 = mybir.dt.float32


def build_nc(n_cores: int = N_CORES, repeats: int = 1):
    nc = bacc.Bacc("TRN2", target_bir_lowering=False, debug=False, num_devices=n_cores)

    xT = nc.dram_tensor("xT", [D, S], BF16, kind="ExternalInput")
    wq = nc.dram_tensor("wq", [D, DH], BF16, kind="ExternalInput")
    wk = nc.dram_tensor("wk", [D, DH], BF16, kind="ExternalInput")
    wv = nc.dram_tensor("wv", [D, DH], BF16, kind="ExternalInput")
    wo = nc.dram_tensor("wo", [DH, D], BF16, kind="ExternalInput")
    bq = nc.dram_tensor("bq", [2, P], F32, kind="ExternalInput")
    bk = nc.dram_tensor("bk", [2, P], F32, kind="ExternalInput")
    bv = nc.dram_tensor("bv", [1, DH], F32, kind="ExternalInput")
    y = nc.dram_tensor("y", [S, D], F32, kind="ExternalOutput")

    with tile.TileContext(nc) as tc:
        for _ in range(repeats):
            _body(nc, tc, xT, wq, wk, wv, wo, bq, bk, bv, y)

    nc.compile()
    return nc


def _body(nc, tc, xT, wq, wk, wv, wo, bq, bk, bv, y):
    add = mybir.AluOpType.add
    Exp = mybir.ActivationFunctionType.Exp

    xT_r = xT.ap().rearrange("(dt p) t -> p dt t", p=P)
    wq_r = wq.ap().rearrange("(dt p) c -> p dt c", p=P)
    wk_r = wk.ap().rearrange("(dt p) c -> p dt c", p=P)
    wv_r = wv.ap().rearrange("(dt p) c -> p dt c", p=P)

    with (
        tc.tile_pool(name="const", bufs=1) as const,
        tc.tile_pool(name="work", bufs=6) as work,
        tc.tile_pool(name="psum", bufs=4, space="PSUM") as psum,
    ):
        # ---- persistent SBUF state (DMA split per d-tile so compute can
        # start as soon as the first slices land) ----
        xT_sb = const.tile([P, DT, S], BF16)
        wq_sb = const.tile([P, DT, DH], BF16)
        wk_sb = const.tile([P, DT, DH], BF16)
        wv_sb = const.tile([P, DT, DH], BF16)
        HS = S // 2
        for dt in range(DT):
            nc.sync.dma_start(wq_sb[:, dt], wq_r[:, dt])
            nc.sync.dma_start(wk_sb[:, dt], wk_r[:, dt])
            nc.sync.dma_start(xT_sb[:, dt, 0:HS], xT_r[:, dt, 0:HS])
        for dt in range(DT):
            nc.sync.dma_start(wv_sb[:, dt], wv_r[:, dt])
            nc.sync.dma_start(xT_sb[:, dt, HS:S], xT_r[:, dt, HS:S])

        wo_sb = const.tile([P, 2, D], BF16)
        nc.sync.dma_start(wo_sb[:], wo.ap().rearrange("(ht p) e -> p ht e", p=P))

        bq_sb = const.tile([P, 2], F32)
        bk_sb = const.tile([P, 2], F32)
        nc.sync.dma_start(bq_sb[:], bq.ap().rearrange("mt p -> p mt"))
        nc.sync.dma_start(bk_sb[:], bk.ap().rearrange("mt p -> p mt"))

        bv_bc = const.tile([P, DH], F32)
        nc.sync.dma_start(bv_bc[0:1, :], bv.ap())
        nc.gpsimd.partition_broadcast(bv_bc[:], bv_bc[0:1, :])

        # causal masks for the 4 diagonal k-tiles of each q-chunk:
        # mask[p, i, ql] = 1.0 if p <= ql - 128*i else 0.0
        mask_sb = const.tile([P, 4, QC], BF16)
        nc.vector.memset(mask_sb[:], 1.0)
        for i in range(4):
            nc.gpsimd.affine_select(
                out=mask_sb[:, i, :],
                in_=mask_sb[:, i, :],
                compare_op=mybir.AluOpType.is_ge,
                fill=0.0,
                base=-P * i,
                pattern=[[1, QC]],
                channel_multiplier=-1,
            )

        qT_sb = const.tile([P, 2, S], BF16)  # [ch within mtile, mtile, t]
        kT_sb = const.tile([P, 2, S], BF16)
        # V' [t-part, ktile, head, dk+1]; col DK holds ones (softmax denom)
        vp_sb = const.tile([P, NKT, HPC, DK + 1], BF16)
        nc.vector.memset(vp_sb[:, :, :, DK : DK + 1], 1.0)
        aT_sb = const.tile([P, 2, S], BF16)  # attention out, channel-major

        # ---- QKV projections ----
        # Q^T / K^T channel-major: psum[ch, t] += w[d, ch].T @ xT[d, t]
        # two 512-wide chunks share a 2-bank psum pair tile -> one copy each
        for wsb, bsb, dst in ((wq_sb, bq_sb, qT_sb), (wk_sb, bk_sb, kT_sb)):
            for mt in range(2):
                for cp in range(NQC // 2):
                    ps = psum.tile([P, 2, QC], F32, tag="pair")
                    for half in range(2):
                        c4 = 2 * cp + half
                        for dt in range(DT):
                            nc.tensor.matmul(
                                ps[:, half, :],
                                lhsT=wsb[:, dt, mt * P : (mt + 1) * P],
                                rhs=xT_sb[:, dt, c4 * QC : (c4 + 1) * QC],
                                start=(dt == 0),
                                stop=(dt == DT - 1),
                            )
                    nc.vector.tensor_scalar(
                        dst[:, mt, 2 * cp * QC : (2 * cp + 2) * QC].rearrange(
                            "p (h q) -> p h q", h=2
                        ),
                        ps[:],
                        bsb[:, mt : mt + 1],
                        None,
                        op0=add,
                    )

        # V token-major: psum[t, j] += xT[d, t-tile].T @ wv[d, j]
        for kt in range(NKT):
            ps = psum.tile([P, 2, QC], F32, tag="pair", name="vps")
            for dt in range(DT):
                nc.tensor.matmul(
                    ps[:, 0, 0:DH],
                    lhsT=xT_sb[:, dt, kt * P : (kt + 1) * P],
                    rhs=wv_sb[:, dt, :],
                    start=(dt == 0),
                    stop=(dt == DT - 1),
                )
            nc.vector.tensor_tensor(
                vp_sb[:, kt, :, 0:DK],
                ps[:, 0, 0:DH].rearrange("p (h j) -> p h j", j=DK),
                bv_bc[:].rearrange("p (h j) -> p h j", j=DK),
                add,
            )

        # ---- attention + interleaved out-proj (lag-1 software pipeline) ----
        # the two heads of a pair share [*, 2, QC] tiles: one exp / mask /
        # division op covers both heads (halves ACT+DVE instruction count)
        def attention(hp, qc):
            nkt = 4 * (qc + 1)  # causal: k-tiles 0 .. 4*qc+3
            av = psum.tile([DK + 1, 2, QC], F32, tag="pair", name="av")
            pend = None  # delay AV by one k-tile to hide exp latency
            for kt in range(nkt):
                diag = kt - 4 * qc  # >= 0 on the 4 diagonal tiles
                cl = max(0, diag) * P  # first valid column of this q-chunk
                sc = psum.tile([P, 2, QC], F32, tag="pair", name="sc")
                for hh in range(2):
                    lo, hi = hh * DK, (hh + 1) * DK
                    nc.tensor.matmul(
                        sc[:, hh, cl:QC],
                        lhsT=kT_sb[lo:hi, hp, kt * P : (kt + 1) * P],
                        rhs=qT_sb[lo:hi, hp, qc * QC + cl : (qc + 1) * QC],
                        start=True,
                        stop=True,
                    )
                ex = work.tile([P, 2, QC], BF16, tag="exp", bufs=8)
                nc.scalar.activation(
                    ex[:, :, cl:QC], sc[:, :, cl:QC], Exp, scale=SCALE
                )
                if diag >= 0:
                    nc.vector.tensor_mul(
                        ex[:, :, cl:QC],
                        ex[:, :, cl:QC],
                        mask_sb[:, diag : diag + 1, cl:QC].to_broadcast(
                            (P, 2, QC - cl)
                        ),
                    )
                if pend is not None:
                    _av_pair(nc, av, vp_sb, hp, pend, qc, last=False)
                pend = (kt, ex)
            _av_pair(nc, av, vp_sb, hp, pend, qc, last=True)

            # softmax division: row DK of av is the denominator; reciprocal
            # on partition DK, DMA the row to partition 0 (gpsimd broadcast
            # only honours base partition 0 on HW), broadcast down, multiply.
            qs = slice(qc * QC, (qc + 1) * QC)
            rec = work.tile([P, 2, QC], F32, tag="rec", bufs=2)
            nc.vector.reciprocal(rec[DK : DK + 1, :, :], av[DK : DK + 1, :, :])
            bcs = work.tile([1, 2, QC], F32, tag="bcs", bufs=2)
            nc.sync.dma_start(bcs[0:1, :, :], rec[DK : DK + 1, :, :])
            bc = work.tile([DK, 2, QC], F32, tag="bc")
            nc.gpsimd.partition_broadcast(bc[:], bcs[0:1, :, :])
            st = work.tile([DK, 2, QC], BF16, tag="st")
            nc.vector.tensor_mul(st[:], av[0:DK, :, :], bc[:])
            nc.sync.dma_start(aT_sb[0:DK, hp, qs], st[:, 0, :])
            nc.sync.dma_start(aT_sb[DK:P, hp, qs], st[:, 1, :])

        def outproj(qc):
            # partial y for t-tiles of chunk qc; b_out is added on the host
            for tt in range(4 * qc, 4 * qc + 4):
                ysb = work.tile([P, D], F32, tag="y")
                ps = psum.tile([P, 2, QC], F32, tag="pair", name="yp")
                for ec in range(2):
                    for ht in range(2):
                        nc.tensor.matmul(
                            ps[:, ec, :],
                            lhsT=aT_sb[:, ht, tt * P : (tt + 1) * P],
                            rhs=wo_sb[:, ht, ec * QC : (ec + 1) * QC],
                            start=(ht == 0),
                            stop=(ht == 1),
                        )
                nc.vector.tensor_copy(ysb[:].rearrange("p (h q) -> p h q", h=2), ps[:])
                nc.sync.dma_start(y.ap()[tt * P : (tt + 1) * P, :], ysb[:])

        qcs = list(range(NQC - 1, -1, -1))
        for i, qc in enumerate(qcs):
            for hp in range(2):
                attention(hp, qc)
            if i > 0:
                outproj(qcs[i - 1])
        outproj(qcs[-1])


def _av_pair(nc, av, vp_sb, hp, pend, qc, last):
    kt, ex = pend
    diag = kt - 4 * qc
    cl = max(0, diag) * P
    for hh in range(2):
        nc.tensor.matmul(
            av[:, hh, cl:QC],
            lhsT=vp_sb[:, kt, 2 * hp + hh, :],
            rhs=ex[:, hh, cl:QC],
            start=(kt == 0),
            stop=last,
        )


def make_core_inputs(x, w_qkv, b_qkv, w_out, b_out):
    """Shard + preprocess full inputs into 8 per-core input dicts."""
    bf16 = ml_dtypes.bfloat16
    x = np.asarray(x, np.float32)
    w_qkv = np.asarray(w_qkv, np.float32)
    b_qkv = np.asarray(b_qkv, np.float32)
    w_out = np.asarray(w_out, np.float32)

    # per-batch transpose+cast computed once and shared by the 4 cores
    xT_cache = [np.ascontiguousarray(x[b].T).astype(bf16) for b in range(B)]
    in_maps = []
    for c in range(N_CORES):
        b, g = divmod(c, 4)
        sl = slice(g * DH, (g + 1) * DH)
        wq = w_qkv[0 * D + g * DH : 0 * D + (g + 1) * DH]  # [DH, D]
        wk = w_qkv[1 * D + g * DH : 1 * D + (g + 1) * DH]
        wv = w_qkv[2 * D + g * DH : 2 * D + (g + 1) * DH]
        in_maps.append(
            {
                "xT": xT_cache[b],
                "wq": np.ascontiguousarray(wq.T).astype(bf16),
                "wk": np.ascontiguousarray(wk.T).astype(bf16),
                "wv": np.ascontiguousarray(wv.T).astype(bf16),
                "wo": np.ascontiguousarray(w_out[:, sl].T).astype(bf16),
                "bq": b_qkv[0 * D + g * DH : 0 * D + (g + 1) * DH]
                .reshape(2, P)
                .astype(np.float32),
                "bk": b_qkv[1 * D + g * DH : 1 * D + (g + 1) * DH]
                .reshape(2, P)
                .astype(np.float32),
                "bv": b_qkv[2 * D + g * DH : 2 * D + (g + 1) * DH]
                .reshape(1, DH)
                .astype(np.float32),
            }
        )
    return in_maps


def gather_output(results, b_out=None):
    """Sum the 4 per-core partials for each batch (+ b_out)."""
    out = np.empty((B, S, D), np.float32)
    for b in range(B):
        acc = results[4 * b]["y"].astype(np.float32)
        for g in range(1, 4):
            acc = acc + results[4 * b + g]["y"]
        out[b] = acc
    if b_out is not None:
        out += np.asarray(b_out, np.float32)
    return out


_NC_CACHE = None


def kernel(x, w_qkv, b_qkv, w_out, b_out):
    global _NC_CACHE
    from concourse.bass_utils import run_bass_kernel_spmd

    if _NC_CACHE is None:
        _NC_CACHE = build_nc()
    in_maps = make_core_inputs(x, w_qkv, b_qkv, w_out, b_out)
    res = run_bass_kernel_spmd(_NC_CACHE, in_maps, core_ids=list(range(N_CORES)))
    return gather_output(res.results, b_out=b_out)
